# revision 14
# baseline (speedup 1.0000x reference)
# Trainium2 Bass kernel for nn_LogitsNew (dense_mlp).
#
#   u = gelu(x @ W_proj + b_proj)                       [B, D]
#   logits = (u @ W_u)[:, None, :] + ee @ W_e           [B, N, C]
#
# Sharding: data-parallel over batch B across 8 cores (4 batches/core).
# All matmuls run in fp16 (1 cycle/row on the PE, FWL weight loads,
# fp32 PSUM accumulation; measured ~3e-4 norm relative error).
# Per core:
#   - inputs are converted fp32 -> fp16 on ACT/DVE; every transpose
#     (ee tiles, x, u) is done by the DMA xbar (InstDmaTransposeAnt),
#     so the PE only executes matmuls.
#   - main path: per 128-row ee tile, accumulate eeT.T @ W_e into two
#     PSUM banks, drain PSUM->SBUF immediately (no y dependency).
#   - utterance path (spliced in after m-tile 3, when its weights have
#     landed): z = x@W_proj (+b via a K=1 ones matmul), u = Gelu(z),
#     y = u@W_u, broadcast y across partitions with gpsimd.
#   - epilogue: out_sb += y_bcast on DVE, DMA out.
#
# DMA rings: SP carries ee[0..3] + all weight slices (W_e, W_proj, W_u in
# consumption order) + stores; ACT carries x/b, ee[4..7], xbar transposes,
# y_row. Engines execute their streams in order, so program order tracks
# data-arrival order. Transposed/converted tiles are private per m-tile
# (no pool-slot reuse) because InstDmaTransposeAnt dependency tracking
# proved unreliable under slot reuse.

import sys

if "/opt/trn_rl_repo" not in sys.path:
    sys.path.insert(0, "/opt/trn_rl_repo")

import numpy as np

import concourse.bass as bass
import concourse.mybir as mybir
import concourse.tile as tile
from concourse import bacc
from concourse.bass_utils import run_bass_kernel_spmd

P = 128
B, N, D, C = 32, 256, 1024, 1024
NCORES = 8
BPC = B // NCORES          # batches per core
KT = D // P                # 8 k-tiles over the contraction dim
FD = 512                   # matmul moving free dim (one PSUM bank of fp32)
NT = N // P                # 2 n-tiles per batch
MT = BPC * NT              # 8 m-tiles per core

F32 = mybir.dt.float32
F16 = mybir.dt.float16
GELU = mybir.ActivationFunctionType.Gelu

_CACHE = {}


def _build():
    if "nc" in _CACHE:
        return _CACHE["nc"]

    nc = bacc.Bacc("TRN2", target_bir_lowering=False, debug=False, num_devices=NCORES)

    x = nc.dram_tensor("encoded_utterance", [BPC, D], F32, kind="ExternalInput").ap()
    ee = nc.dram_tensor(
        "element_embeddings", [BPC, N, D], F32, kind="ExternalInput"
    ).ap()
    w = nc.dram_tensor("weight_matrix", [2 * D, C], F32, kind="ExternalInput").ap()
    wp = nc.dram_tensor("W_proj", [D, D], F32, kind="ExternalInput").ap()
    bp = nc.dram_tensor("b_proj", [1, D], F32, kind="ExternalInput").ap()
    out = nc.dram_tensor("logits", [BPC, N, C], F32, kind="ExternalOutput").ap()

    w3 = w.rearrange("(ko p) c -> p ko c", p=P)     # [128, 16, 1024]; ko 0..7 = W_u
    wp3 = wp.rearrange("(ko p) c -> p ko c", p=P)   # [128, 8, 1024]

    with tile.TileContext(nc) as tc:
        with (
            tc.tile_pool(name="const", bufs=1) as cpool,
            tc.tile_pool(name="weights", bufs=1) as wpool,
            tc.tile_pool(name="westage", bufs=2) as wspool,
            tc.tile_pool(name="ee", bufs=2) as eepool,
            tc.tile_pool(name="eebf", bufs=1) as eebfpool,
            tc.tile_pool(name="eet", bufs=1) as eetpool,
            tc.tile_pool(name="outs", bufs=1) as outpool,
            tc.tile_pool(name="mm_ps", bufs=8, space="PSUM") as mm_ps,
        ):
            # ---- small inputs (ACT ring) + fp16 conversions ----
            ones = cpool.tile([1, P], F16)
            nc.gpsimd.memset(ones, 1.0)
            x_f = cpool.tile([BPC, D], F32)
            nc.scalar.dma_start(x_f, x)
            b_f = cpool.tile([1, D], F32)
            nc.scalar.dma_start(b_f, bp)
            # x16 padded to 32 partitions for the xbar transpose
            x16 = cpool.tile([32, D], F16)
            nc.scalar.copy(x16[:BPC], x_f)
            b16 = cpool.tile([1, D], F16)
            nc.scalar.copy(b16, b_f)

            # ---- first 4 ee tiles on the SP ring, ahead of the weights ----
            ee_tiles = {}
            for mt in range(4):
                b, nh = divmod(mt, NT)
                ee_t = eepool.tile([P, D], F32, tag="ee", name=f"ee_{mt}")
                nc.sync.dma_start(ee_t, ee[b, nh * P : (nh + 1) * P, :])
                ee_tiles[mt] = ee_t

            # ---- weights on the SP ring, 1MB slices, in consumption order,
            # staged then converted to fp16 (ACT/DVE alternating). ----
            we16 = wpool.tile([P, KT, C], F16)
            wp16 = wpool.tile([P, KT, C], F16)
            wu16 = wpool.tile([P, KT, C], F16)
            for i, (dst, src) in enumerate(
                [(we16, w3[:, 8:]), (wp16, wp3), (wu16, w3[:, :8])]
            ):
                for j in range(4):
                    wes = wspool.tile(
                        [P, 2, C], F32, tag="wes", name=f"wes_{i}_{j}"
                    )
                    nc.sync.dma_start(wes, src[:, 2 * j : 2 * j + 2])
                    if (i * 4 + j) % 2 == 0:
                        nc.scalar.copy(dst[:, 2 * j : 2 * j + 2, :], wes)
                    else:
                        nc.vector.tensor_copy(dst[:, 2 * j : 2 * j + 2, :], wes)

            # ---- main path (utterance path spliced in after m-tile 3) ----
            out_tiles = []
            for mt in range(MT):
                if mt == 4:
                    # ---- utterance path ----
                    xT = cpool.tile([P, KT, 32], F16)
                    nc.scalar.dma_start_transpose(xT, x16)

                    u16 = cpool.tile([32, C], F16)
                    for h in range(2):
                        cs = slice(h * FD, (h + 1) * FD)
                        zp = mm_ps.tile([P, FD], F32, tag="mm", name=f"z_{h}")
                        for k in range(KT):
                            nc.tensor.matmul(
                                zp[:BPC], xT[:, k, :BPC], wp16[:, k, cs],
                                start=(k == 0), stop=False,
                            )
                        nc.tensor.matmul(
                            zp[:BPC], ones[:1, :BPC], b16[:1, cs],
                            start=False, stop=True,
                        )
                        nc.scalar.activation(u16[:BPC, cs], zp[:BPC], GELU)

                    uT = cpool.tile([P, KT, 32], F16)
                    nc.scalar.dma_start_transpose(uT, u16)

                    y_sb = cpool.tile([BPC, C], F32)
                    for h in range(2):
                        cs = slice(h * FD, (h + 1) * FD)
                        yp = mm_ps.tile([P, FD], F32, tag="mm", name=f"y_{h}")
                        for k in range(KT):
                            nc.tensor.matmul(
                                yp[:BPC], uT[:, k, :BPC], wu16[:, k, cs],
                                start=(k == 0), stop=(k == KT - 1),
                            )
                        nc.vector.tensor_copy(y_sb[:, cs], yp[:BPC])

                    y_row = cpool.tile([1, BPC, C], F32)
                    nc.scalar.dma_start(y_row, y_sb)
                    ybc = cpool.tile([P, BPC, C], F32)
                    for b2 in range(BPC):
                        nc.gpsimd.partition_broadcast(ybc[:, b2, :], y_row[:1, b2, :])

                b, nh = divmod(mt, NT)
                ns = slice(nh * P, (nh + 1) * P)
                if mt >= 4:
                    ee_t = eepool.tile([P, D], F32, tag="ee", name=f"ee_{mt}")
                    nc.scalar.dma_start(ee_t, ee[b, ns, :])
                    ee_tiles[mt] = ee_t
                ee_t = ee_tiles[mt]
                # fp32 -> fp16 (halves on ACT and DVE in parallel)
                ee_bf = eebfpool.tile([P, D], F16, tag=f"eebf{mt}", name=f"eebf_{mt}")
                nc.scalar.copy(ee_bf[:, : D // 2], ee_t[:, : D // 2])
                nc.vector.tensor_copy(ee_bf[:, D // 2 :], ee_t[:, D // 2 :])
                # all 8 [128,128] sub-transposes in one xbar DMA (ACT ring)
                eet = eetpool.tile([P, KT, P], F16, tag=f"eet{mt}", name=f"eet_{mt}")
                nc.scalar.dma_start_transpose(eet, ee_bf)
                mps = [
                    mm_ps.tile([P, FD], F32, tag="mm", name=f"mm_{mt}_{ch}")
                    for ch in range(2)
                ]
                for ch in range(2):
                    for k in range(KT):
                        nc.tensor.matmul(
                            mps[ch],
                            eet[:, k, :],
                            we16[:, k, ch * FD : (ch + 1) * FD],
                            start=(k == 0),
                            stop=(k == KT - 1),
                        )
                o = outpool.tile([P, 2, FD], F32, tag=f"o{mt}")
                nc.scalar.copy(o[:, 0, :], mps[0])
                nc.scalar.copy(o[:, 1, :], mps[1])
                out_tiles.append(o)

            # ---- epilogue: add broadcast y, store ----
            for mt in range(MT):
                b, nh = divmod(mt, NT)
                ns = slice(nh * P, (nh + 1) * P)
                o = out_tiles[mt]
                nc.vector.tensor_add(o[:, 0, :], o[:, 0, :], ybc[:, b, 0:FD])
                nc.vector.tensor_add(o[:, 1, :], o[:, 1, :], ybc[:, b, FD:C])
                nc.sync.dma_start(out[b, ns, :], o.rearrange("p a f -> p (a f)"))

    nc.compile()
    _CACHE["nc"] = nc
    return nc


def run(inputs, trace=False, **kwargs):
    nc = _build()
    x = np.ascontiguousarray(np.asarray(inputs["encoded_utterance"], np.float32))
    ee = np.ascontiguousarray(np.asarray(inputs["element_embeddings"], np.float32))
    w = np.ascontiguousarray(np.asarray(inputs["weight_matrix"], np.float32))
    wp = np.ascontiguousarray(np.asarray(inputs["W_proj"], np.float32))
    bp = np.ascontiguousarray(
        np.asarray(inputs["b_proj"], np.float32).reshape(1, D)
    )

    in_maps = []
    for i in range(NCORES):
        bs = slice(i * BPC, (i + 1) * BPC)
        in_maps.append(
            {
                "encoded_utterance": x[bs],
                "element_embeddings": ee[bs],
                "weight_matrix": w,
                "W_proj": wp,
                "b_proj": bp,
            }
        )

    res = run_bass_kernel_spmd(
        nc, in_maps, core_ids=list(range(NCORES)), trace=trace, **kwargs
    )
    full = np.concatenate([r["logits"] for r in res.results], axis=0)
    return full, res


def kernel(**inputs) -> np.ndarray:
    return run(inputs, trace=False)[0]
